# revision 63
# baseline (speedup 1.0000x reference)
"""Distributed GQA attention kernel for 8 TRN2 NeuronCores.

Sharding (tensor-parallel over heads): core i owns q-heads [8i, 8i+8) and
kv-head i (GQA n_rep=8, so one kv head serves all 8 local q heads).

Pipeline (single pass, no barriers; Tile sems carry all deps):
  w1: QKV projection chunk sc0 (q/k/v positions 0:512) + v-transpose
  w2: QKV sc1 interleaved with attention chunk qc0 (queries 0:512)
  w3: QKV sc2 interleaved with attention qc1 + wo(qc0) + ReduceScatter(qc0)
  w4: attention qc2 interleaved with wo(qc1) + RS(qc1)
  w5: wo(qc2) + RS(qc2)

Engine-level structure:
  - RoPE in bf16: PSUM evicted on Scalar (Identity+bias), cos/sin muls on
    Vector in bf16 (sin term uses partition-shifted reads, no rotate copy).
  - Scores per 128-key block in [k, q] layout; exp on Scalar (bf16 out),
    causal diag masked by a bf16 triangle multiply on Vector.
  - AV in transposed orientation: poT[d, q] accumulates one matmul per key
    block (vaug stationary); an identity-row init matmul seeds row 64 with
    exp(sink) so the denominator rides the ones column.
  - Normalization: reciprocal of poT row 64 (Vector), partition_broadcast
    (GpSimd), one multiply into oTt (Vector) -- lands pre-transposed for wo.
  - RS outputs are kernel outputs (bf16); host upcasts and reassembles.
"""

import contextlib
import ctypes
import os
import sys

import numpy as np

sys.path.insert(0, "/opt/trn_rl_repo")

S = 1536
HID = 2880
D = 64
HL = 8          # local q heads per core
CORES = 8
SCQ = 512       # QKV moving chunk
NSC = S // SCQ
QC = 512        # attention q chunk
NQC = S // QC
KBN = S // 128  # 12 k blocks
VA = 68         # poT partition count: 64 dims + ones rows
CBF = 22        # full 128-row contraction blocks (2880 = 22*128 + 64)
MBN = 23        # wo output row blocks (22 full + one 64)
JBN = 4         # 512 local j rows = 4 blocks

_EXEC_TIME_NS = [None]


def _install_hooks():
    import types

    import antenv

    try:
        from antenv import axon_hooks
    except ImportError:
        axon_hooks = types.ModuleType("antenv.axon_hooks")
        _holder = {"hook": None}
        axon_hooks.set_axon_ntff_profile_hook = lambda h: _holder.update(hook=h)
        axon_hooks.get_axon_ntff_profile_hook = lambda: _holder["hook"]
        sys.modules["antenv.axon_hooks"] = axon_hooks
        antenv.axon_hooks = axon_hooks

    so_path = "/opt/axon/libaxon_pjrt.so"
    hook = None
    if os.path.exists(so_path):
        lib = ctypes.CDLL(so_path)
        if hasattr(lib, "axon_start_nrt_profile"):
            lib.axon_start_nrt_profile.argtypes = [
                ctypes.POINTER(ctypes.c_int64),
                ctypes.c_size_t,
            ]
            lib.axon_start_nrt_profile.restype = ctypes.c_int64
            lib.axon_stop_nrt_profile.argtypes = [ctypes.c_char_p]
            lib.axon_stop_nrt_profile.restype = ctypes.c_int64

            @contextlib.contextmanager
            def hook(output_dir, device_ids):
                import jax

                jax.devices()
                if device_ids:
                    ids = (ctypes.c_int64 * len(device_ids))(*device_ids)
                    rc = lib.axon_start_nrt_profile(ids, len(device_ids))
                else:
                    rc = lib.axon_start_nrt_profile(None, 0)
                if rc != 0:
                    raise RuntimeError(f"axon_start_nrt_profile rc={rc}")
                try:
                    yield
                finally:
                    n = lib.axon_stop_nrt_profile(str(output_dir).encode())
                    print(f"profile: {n} file(s) written to {output_dir}")

    axon_hooks.set_axon_ntff_profile_hook(hook)

    import concourse.bass_utils as bu

    bu.upload_artifacts = lambda tmpdir: f"local://{tmpdir}"

    if os.environ.get("BASS_LDW_OPT", "0") == "1" and not getattr(
        bu, "_ldw_patched", False
    ):
        _orig_run = bu.run_command

        def _run(cmd, *a, **k):
            cmd = [
                c.replace("--enable-ldw-opt=false", "--enable-ldw-opt=true")
                if isinstance(c, str)
                else c
                for c in cmd
            ]
            return _orig_run(cmd, *a, **k)

        bu.run_command = _run
        bu._ldw_patched = True


def build_graph():
    import concourse.mybir as mybir
    import concourse.tile as tile
    from concourse import bacc
    from concourse.masks import make_identity

    F32 = mybir.dt.float32
    BF16 = mybir.dt.bfloat16
    Exp = mybir.ActivationFunctionType.Exp
    Ident = mybir.ActivationFunctionType.Identity

    nc = bacc.Bacc("TRN2", target_bir_lowering=False, debug=False, num_devices=CORES)

    xT = nc.declare_dram_parameter("xT", [HID, S], BF16, isOutput=False)
    wT = nc.declare_dram_parameter("wT", [HID, 640], BF16, isOutput=False)
    bq = nc.declare_dram_parameter("bq", [64, HL], F32, isOutput=False)
    bk = nc.declare_dram_parameter("bk", [64, 1], F32, isOutput=False)
    bv = nc.declare_dram_parameter("bv", [64, 1], F32, isOutput=False)
    cosb = nc.declare_dram_parameter("cosb", [64, S], BF16, isOutput=False)
    sinbs = nc.declare_dram_parameter("sinbs", [64, S], BF16, isOutput=False)
    woT = nc.declare_dram_parameter("woT", [512, HID], BF16, isOutput=False)
    wob8 = nc.declare_dram_parameter("wob8", [128, MBN], F32, isOutput=False)
    esinkS = nc.declare_dram_parameter("esinkS", [1, HL], F32, isOutput=False)
    outs = [
        nc.declare_dram_parameter(f"o{qc}", [360, QC], BF16, isOutput=True)
        for qc in range(2)
    ]
    out2a = nc.declare_dram_parameter("o2a", [176, QC], BF16, isOutput=True)
    out2b = nc.declare_dram_parameter("o2b", [184, QC], BF16, isOutput=True)
    yT_part = [nc.dram_tensor(f"yT_part_{qc}", [HID, QC], BF16) for qc in range(NQC)]
    yT_red = [nc.dram_tensor(f"yT_red_{qc}", [360, QC], BF16) for qc in range(2)]
    yT_red2a = nc.dram_tensor("yT_red_2a", [176, QC], BF16)
    yT_red2b = nc.dram_tensor("yT_red_2b", [184, QC], BF16)
    warm_in = nc.dram_tensor("warm_in", [8, 16], BF16)
    warm_out = nc.dram_tensor("warm_out", [1, 16], BF16)

    # QKV contraction pieces: groups of 128-row c-blocks (22 full + one 64-row)
    # first pieces kept small so the very first matmul's operands land fast
    PIECES = [(0, 1), (1, 1), (2, 2), (4, 4), (8, 4), (12, 4), (16, 4), (20, 2)]

    with tile.TileContext(nc) as tc:
        with contextlib.ExitStack() as stack:
            consts = stack.enter_context(tc.tile_pool(name="consts", bufs=1))
            qkvout = stack.enter_context(tc.tile_pool(name="qkvout", bufs=1))
            esp = stack.enter_context(tc.tile_pool(name="esp", bufs=6))
            small = stack.enter_context(tc.tile_pool(name="small", bufs=2))
            ytp = stack.enter_context(tc.tile_pool(name="ytp", bufs=1))
            rtmp = stack.enter_context(tc.tile_pool(name="rtmp", bufs=2))
            oTp = stack.enter_context(tc.tile_pool(name="oTp", bufs=1))
            woTp = stack.enter_context(tc.tile_pool(name="woTp", bufs=1))
            scps = stack.enter_context(
                tc.tile_pool(name="scps", bufs=3, space="PSUM")
            )
            pops = stack.enter_context(
                tc.tile_pool(name="pops", bufs=2, space="PSUM")
            )

            bqt = consts.tile([64, HL], F32, tag="bq")
            bkt = consts.tile([64, 1], F32, tag="bk")
            bvt = consts.tile([64, 1], F32, tag="bv")
            cost = consts.tile([64, S], BF16, tag="cos")
            sint = consts.tile([64, S], BF16, tag="sin")
            wob8t = consts.tile([128, MBN], F32, tag="wob8")
            eskt = consts.tile([1, HL], F32, tag="esinkS")
            ident_f = consts.tile([128, 128], F32, tag="ident_f")
            ident = consts.tile([128, 128], BF16, tag="ident")
            for t, src_ in [(bqt, bq), (bkt, bk), (bvt, bv), (cost, cosb),
                            (sint, sinbs), (wob8t, wob8), (eskt, esinkS)]:
                nc.sync.dma_start(out=t[:, :], in_=src_[:, :])
            make_identity(nc, ident_f[:, :])
            nc.vector.tensor_copy(ident[:, :], ident_f[:, :])
            tri = consts.tile([128, 128], BF16, tag="tri")
            nc.vector.memset(tri[:, :], 1.0)
            nc.gpsimd.affine_select(
                out=tri[:, :], in_=tri[:, :],
                compare_op=mybir.AluOpType.is_ge,
                fill=0.0, base=0, pattern=[[1, 128]], channel_multiplier=-1,
            )

            qq = qkvout.tile([64, HL * S], BF16, tag="qq")
            kh = qkvout.tile([64, S], BF16, tag="kh")
            vT = qkvout.tile([64, S], BF16, tag="vT")
            vaug = qkvout.tile([128, KBN * VA], BF16, tag="vaug")
            nc.vector.memset(vaug[:, :], 1.0)

            oTt = oTp.tile([128, JBN * S], BF16, tag="oT")
            woTt = woTp.tile([128, JBN * HID], BF16, tag="woT")

            # ---------- emitters ----------

            def rope64(dst, p_ap, bias_ap, c0, w, name):
                """PSUM [64,w] -> dst bf16 with bias + RoPE (rotate-half)."""
                qb = rtmp.tile([64, SCQ], BF16, tag="qb", name=f"qb_{name}")
                nc.scalar.activation(qb[:, 0:w], p_ap, Ident, bias=bias_ap)
                rot = rtmp.tile([64, SCQ], BF16, tag="rot", name=f"rt_{name}")
                nc.vector.tensor_copy(rot[0:32, 0:w], qb[32:64, 0:w])
                nc.vector.tensor_copy(rot[32:64, 0:w], qb[0:32, 0:w])
                term = rtmp.tile([64, SCQ], BF16, tag="term", name=f"tm_{name}")
                # term = rot_half(qb) * sin  (sign folded into sint host-side)
                nc.vector.tensor_mul(term[:, 0:w], rot[:, 0:w], sint[:, c0:c0 + w])
                qcos = rtmp.tile([64, SCQ], BF16, tag="qcos", name=f"qc_{name}")
                nc.vector.tensor_mul(qcos[:, 0:w], qb[:, 0:w], cost[:, c0:c0 + w])
                nc.vector.tensor_add(dst, qcos[:, 0:w], term[:, 0:w])

            wts = []
            xcs = {}

            def emit_w_piece(wtp, pc):
                cb0, ncb = PIECES[pc]
                wt_pc = wtp.tile([128, ncb * 640], BF16, tag=f"wt{pc}",
                                 name=f"wt{pc}")
                nc.scalar.dma_start(
                    out=wt_pc[:, :].rearrange("p (cb n) -> p cb n", cb=ncb),
                    in_=wT[cb0 * 128:(cb0 + ncb) * 128, :].rearrange(
                        "(cb p) n -> p cb n", p=128
                    ),
                )
                wts.append(wt_pc)

            def emit_w_tail(wtp):
                wt2 = wtp.tile([64, 640], BF16, tag="wtail")
                nc.sync.dma_start(out=wt2[:, :], in_=wT[CBF * 128:HID, :])
                wts.append(wt2)

            def emit_x_piece(xcp, sc, pc):
                c0 = sc * SCQ
                cb0, ncb = PIECES[pc]
                xp = xcp.tile([128, ncb * SCQ], BF16, tag=f"xp{pc}",
                              name=f"xp{pc}_{sc}")
                nc.sync.dma_start(
                    out=xp[:, :].rearrange("p (cb s) -> p cb s", cb=ncb),
                    in_=xT[cb0 * 128:(cb0 + ncb) * 128, c0:c0 + SCQ].rearrange(
                        "(cb p) s -> p cb s", p=128
                    ),
                )
                xcs.setdefault(sc, []).append(xp)

            def emit_x_tail(xcp, sc):
                c0 = sc * SCQ
                xc2 = xcp.tile([64, SCQ], BF16, tag="xc2", name=f"xc2_{sc}")
                nc.sync.dma_start(out=xc2[:, :], in_=xT[CBF * 128:HID, c0:c0 + SCQ])
                xcs[sc].append(xc2)

            def emit_x_dma(xcp, sc):
                for pc in range(len(PIECES)):
                    emit_x_piece(xcp, sc, pc)
                emit_x_tail(xcp, sc)

            def emit_qkv_nb(qkps, sc, nb):
                c0 = sc * SCQ
                xps = xcs[sc]
                p = qkps.tile([128, SCQ], F32, tag="qkv", name=f"qk{nb}_{sc}")
                for pc, (cb0, ncb) in enumerate(PIECES):
                    for i in range(ncb):
                        nc.tensor.matmul(
                            p[:, :],
                            wts[pc][:, i * 640 + nb * 128:i * 640 + (nb + 1) * 128],
                            xps[pc][:, i * SCQ:(i + 1) * SCQ],
                            start=(pc == 0 and i == 0),
                            stop=False,
                        )
                nc.tensor.matmul(
                    p[:, :],
                    wts[len(PIECES)][:, nb * 128:(nb + 1) * 128],
                    xps[len(PIECES)][:, :],
                    start=False, stop=True,
                )
                if nb < 4:
                    for half in range(2):
                        h = 2 * nb + half
                        hb = 64 * half
                        rope64(
                            qq[:, h * S + c0:h * S + c0 + SCQ],
                            p[hb:hb + 64, :], bqt[:, h:h + 1], c0, SCQ,
                            f"q{sc}_{nb}_{half}",
                        )
                else:
                    rope64(
                        kh[:, c0:c0 + SCQ], p[0:64, :], bkt[:, 0:1], c0, SCQ,
                        f"k{sc}",
                    )
                    nc.scalar.activation(
                        vT[:, c0:c0 + SCQ], p[64:128, :], Ident, bias=bvt[:, 0:1]
                    )

            def emit_v_transpose(vtp, sc):
                pv = vtp.tile([128, 4 * D], BF16, tag="pv", name=f"pv{sc}")
                for j in range(4):
                    kb = sc * 4 + j
                    nc.tensor.transpose(
                        pv[:, j * D:(j + 1) * D],
                        vT[:, kb * 128:(kb + 1) * 128],
                        ident[0:64, 0:64],
                    )
                for j in range(4):
                    kb = sc * 4 + j
                    nc.vector.tensor_copy(
                        vaug[:, kb * VA:kb * VA + D], pv[:, j * D:(j + 1) * D]
                    )

            # attention head state for deferred finalize
            pend = [None]

            def _finalize_head(qc, h, poT):
                q0 = qc * QC
                # denominator row + exp(sink) folded into the PSUM eviction
                dn = small.tile([1, QC], F32, tag="dn", name=f"dn_{qc}_{h}")
                nc.vector.tensor_scalar_add(
                    dn[:, :], poT[64:65, :], eskt[0:1, h:h + 1]
                )
                rcp = small.tile([1, QC], F32, tag="rcp", name=f"rcp_{qc}_{h}")
                nc.vector.reciprocal_approx_fast(out=rcp[:, :], in_=dn[:, :])
                rb = small.tile([64, QC], F32, tag="rb", name=f"rb_{qc}_{h}")
                nc.gpsimd.partition_broadcast(rb[:, :], rcp[0:1, :], channels=64)
                jb, ro = h // 2, (h % 2) * 64
                if ro == 0:
                    nc.vector.tensor_mul(
                        oTt[0:64, jb * S + q0:jb * S + q0 + QC],
                        poT[0:64, :], rb[:, :],
                    )
                else:
                    on = small.tile([64, QC], BF16, tag="on", name=f"on_{qc}_{h}")
                    nc.vector.tensor_mul(on[:, :], poT[0:64, :], rb[:, :])
                    nc.vector.tensor_copy(
                        oTt[64:128, jb * S + q0:jb * S + q0 + QC], on[:, :]
                    )

            def emit_finalize():
                if pend[0] is None:
                    return
                qc, hA, poTA, hB, poTB = pend[0]
                pend[0] = None
                _finalize_head(qc, hA, poTA)
                _finalize_head(qc, hB, poTB)

            def emit_attn_pair(qc, pr, filler=None):
                """Heads hA=2*pr, hB=2*pr+1 together, kb-major so the kh/vaug
                stationaries are loaded once per kb for both heads."""
                hA, hB = 2 * pr, 2 * pr + 1
                q0 = qc * QC
                qb0 = q0 // 128
                nkb = qb0 + 4
                es_t = {hA: [None] * nkb, hB: [None] * nkb}
                poTs = {
                    h: pops.tile([VA, QC], F32, tag="poT", name=f"poT_{qc}_{h}")
                    for h in (hA, hB)
                }

                def emit_scores(h, kb):
                    j = kb - qb0
                    w0 = 128 * j if j > 0 else 0
                    w = QC - w0
                    ps_s = scps.tile([128, QC], F32, tag="sc",
                                     name=f"ps_{qc}_{h}_{kb}")
                    nc.tensor.matmul(
                        ps_s[:, 0:w],
                        kh[:, kb * 128:(kb + 1) * 128],
                        qq[:, h * S + q0 + w0:h * S + q0 + QC],
                        start=True, stop=True,
                    )
                    es = esp.tile([128, QC], BF16, tag="es",
                                  name=f"es_{qc}_{h}_{kb}")
                    nc.scalar.activation(es[:, 0:w], ps_s[:, 0:w], Exp, scale=0.125)
                    if j >= 0:
                        nc.vector.tensor_mul(es[:, 0:128], es[:, 0:128], tri[:, :])
                    es_t[h][kb] = es

                def emit_av(h, kb):
                    j = kb - qb0
                    es = es_t[h][kb]
                    poT = poTs[h]
                    first = kb == 0
                    if j <= 0:
                        nc.tensor.matmul(
                            poT[:, :], vaug[:, kb * VA:(kb + 1) * VA],
                            es[:, 0:QC], start=first, stop=(j == 0),
                        )
                    else:
                        w0 = 128 * j
                        w = QC - w0
                        nc.tensor.matmul(
                            poT[:, w0:w0 + 128],
                            vaug[:, kb * VA:(kb + 1) * VA],
                            es[:, 0:128], start=first, stop=True,
                        )
                        if w > 128:
                            nc.tensor.matmul(
                                poT[:, w0 + 128:QC],
                                vaug[:, kb * VA:(kb + 1) * VA],
                                es[:, 128:w], start=first, stop=False,
                            )

                LAG = 2
                for kb in range(nkb):
                    emit_scores(hA, kb)
                    emit_scores(hB, kb)
                    if kb == 1:
                        emit_finalize()
                    if kb >= LAG:
                        emit_av(hA, kb - LAG)
                        emit_av(hB, kb - LAG)
                    if filler is not None:
                        filler()
                for kb in range(max(0, nkb - LAG), nkb):
                    emit_av(hA, kb)
                    emit_av(hB, kb)
                pend[0] = (qc, hA, poTs[hA], hB, poTs[hB])

            # one big staging tile: wo evictions land here and each chunk is
            # shipped to DRAM with a single DMA, so compute never blocks on
            # per-block DMA completion while a ReduceScatter hogs the DMA
            # engines
            yt_big = ytp.tile([128, MBN * QC], BF16, tag="ytbig")

            def emit_wo_block(wps, qc, mb, evict_engine="vector", dma=True):
                q0 = qc * QC
                rows = 128 if mb < CBF else 64
                pw = wps.tile([128, QC], F32, tag="qkv", name=f"pw_{qc}_{mb}")
                for jb in range(JBN):
                    nc.tensor.matmul(
                        pw[0:rows, :],
                        woTt[:, jb * HID + mb * 128:jb * HID + mb * 128 + rows],
                        oTt[:, jb * S + q0:jb * S + q0 + QC],
                        start=(jb == 0), stop=(jb == JBN - 1),
                    )
                yt = yt_big[:, mb * QC:(mb + 1) * QC]
                if evict_engine == "vector":
                    nc.vector.tensor_scalar_add(
                        yt[0:rows, :], pw[0:rows, :], wob8t[0:rows, mb:mb + 1]
                    )
                else:
                    nc.scalar.activation(
                        yt[0:rows, :], pw[0:rows, :], Ident,
                        bias=wob8t[0:rows, mb:mb + 1],
                    )
                if dma and mb == MBN - 1:
                    nc.sync.dma_start(
                        out=yT_part[qc][0:CBF * 128, :].rearrange(
                            "(mb p) q -> p mb q", p=128
                        ),
                        in_=yt_big[:, 0:CBF * QC].rearrange(
                            "p (mb q) -> p mb q", mb=CBF
                        ),
                    )
                    nc.sync.dma_start(
                        out=yT_part[qc][CBF * 128:HID, :],
                        in_=yt_big[0:64, CBF * QC:MBN * QC],
                    )

            def emit_rs(qc):
                nc.gpsimd.collective_compute(
                    "ReduceScatter",
                    mybir.AluOpType.add,
                    replica_groups=[list(range(CORES))],
                    ins=[yT_part[qc].ap().opt()],
                    outs=[yT_red[qc].ap().opt()],
                )

            # ---------- emission schedule ----------
            with (
                tc.tile_pool(name="wtp", bufs=1) as wtp,
                tc.tile_pool(name="xcp", bufs=2) as xcp,
                tc.tile_pool(name="qkps", bufs=2, space="PSUM") as qkps,
                tc.tile_pool(name="vtp", bufs=1, space="PSUM") as vtp,
            ):
                # tiny warm-up collective: absorbs the cores' initial arrival
                # skew during QKV so RS(0) doesn't pay it later
                nc.gpsimd.collective_compute(
                    "ReduceScatter",
                    mybir.AluOpType.add,
                    replica_groups=[list(range(CORES))],
                    ins=[warm_in.ap().opt()],
                    outs=[warm_out.ap().opt()],
                )
                # interleave x(sc0) and w piece loads so nb0's operands land
                # first and compute starts ASAP
                for pc in range(len(PIECES)):
                    emit_x_piece(xcp, 0, pc)
                    emit_w_piece(wtp, pc)
                emit_x_tail(xcp, 0)
                emit_w_tail(wtp)
                # w1: QKV chunk 0
                emit_x_dma(xcp, 1)
                # woT is first needed by wo(0) in w3; don't block startup DMAs
                nc.scalar.dma_start(
                    out=woTt[:, :].rearrange("p (jb m) -> p jb m", jb=JBN),
                    in_=woT[:, :].rearrange("(jb p) m -> p jb m", p=128),
                )
                for nb in range(5):
                    emit_qkv_nb(qkps, 0, nb)
                emit_v_transpose(vtp, 0)
                # w2: QKV chunk 1 interleaved with attention qc0
                emit_x_dma(xcp, 2)
                for pr in range(4):
                    emit_qkv_nb(qkps, 1, pr)
                    emit_attn_pair(0, pr)
                emit_qkv_nb(qkps, 1, 4)
                emit_v_transpose(vtp, 1)
                # w3: QKV chunk 2 + attention qc1 + wo(0) + RS(0)
                wo0 = [0]

                def fill0():
                    if wo0[0] < MBN:
                        emit_wo_block(qkps, 0, wo0[0])
                        wo0[0] += 1
                        if wo0[0] == MBN:
                            emit_rs(0)

                for pr in range(4):
                    emit_attn_pair(1, pr, filler=fill0 if pr > 0 else None)
                    if pr < 2:
                        emit_qkv_nb(qkps, 2, 2 * pr)
                        emit_qkv_nb(qkps, 2, 2 * pr + 1)
                    elif pr == 2:
                        emit_qkv_nb(qkps, 2, 4)
                while wo0[0] < MBN:
                    fill0()
                emit_v_transpose(vtp, 2)

            with tc.tile_pool(name="wops", bufs=2, space="PSUM") as wops:
                # w4: attention qc2 + wo(1) + RS(1), wo(1) threaded as filler
                wo1 = [0]

                def fill1():
                    if wo1[0] < MBN:
                        emit_wo_block(wops, 1, wo1[0])
                        wo1[0] += 1
                        if wo1[0] == MBN:
                            emit_rs(1)

                for pr in range(4):
                    emit_attn_pair(2, pr, filler=fill1 if pr > 0 else None)
                emit_finalize()
                while wo1[0] < MBN:
                    fill1()
                # w5: wo(2) split in two row-halves so RS(2a) overlaps the
                # second half's compute; scalar is idle here so it shares
                # evictions
                for mb in range(MBN):
                    emit_wo_block(wops, 2, mb,
                                  evict_engine="scalar" if mb % 2 else "vector",
                                  dma=False)
                    if mb == 10:
                        nc.sync.dma_start(
                            out=yT_part[2][0:1408, :].rearrange(
                                "(mb p) q -> p mb q", p=128
                            ),
                            in_=yt_big[:, 0:11 * QC].rearrange(
                                "p (mb q) -> p mb q", mb=11
                            ),
                        )
                        nc.gpsimd.collective_compute(
                            "ReduceScatter",
                            mybir.AluOpType.add,
                            replica_groups=[list(range(CORES))],
                            ins=[yT_part[2][0:1408, :].opt()],
                            outs=[yT_red2a.ap().opt()],
                        )
                nc.sync.dma_start(
                    out=yT_part[2][1408:CBF * 128, :].rearrange(
                        "(mb p) q -> p mb q", p=128
                    ),
                    in_=yt_big[:, 11 * QC:CBF * QC].rearrange(
                        "p (mb q) -> p mb q", mb=CBF - 11
                    ),
                )
                nc.sync.dma_start(
                    out=yT_part[2][CBF * 128:HID, :],
                    in_=yt_big[0:64, CBF * QC:MBN * QC],
                )
                nc.gpsimd.collective_compute(
                    "ReduceScatter",
                    mybir.AluOpType.add,
                    replica_groups=[list(range(CORES))],
                    ins=[yT_part[2][1408:HID, :].opt()],
                    outs=[yT_red2b.ap().opt()],
                )
                # final DRAM->DRAM copies into output params; the sync queue
                # only carries the chunk-level yt_big DMAs mid-kernel, each
                # issued after the RS the corresponding copy waits on, so
                # these can't block anything even if the scheduler hoists them
                for qc in range(2):
                    nc.sync.dma_start(out=outs[qc][:, :], in_=yT_red[qc][:, :])
                nc.sync.dma_start(out=out2a[:, :], in_=yT_red2a[:, :])
                nc.sync.dma_start(out=out2b[:, :], in_=yT_red2b[:, :])

    nc.finalize()
    return nc


def make_in_maps(x, rope_cache, wq_w, wq_b, wk_w, wk_b, wv_w, wv_b, wo_w, wo_b, sinks):
    import ml_dtypes

    BF = ml_dtypes.bfloat16
    xT = np.ascontiguousarray(x[0].T).astype(BF)  # [2880, 1536]
    cosb = np.ascontiguousarray(rope_cache[:, :D].T).astype(BF)
    sinT = np.ascontiguousarray(rope_cache[:, D:].T, dtype=np.float32)
    sinT[: D // 2] *= -1.0
    sinbs = sinT.astype(BF)

    in_maps = []
    for i in range(CORES):
        wq = wq_w[512 * i:512 * (i + 1)]
        wk = wk_w[64 * i:64 * (i + 1)]
        wv = wv_w[64 * i:64 * (i + 1)]
        wT = np.ascontiguousarray(np.concatenate([wq, wk, wv], axis=0).T).astype(BF)
        bq = np.ascontiguousarray(
            wq_b[512 * i:512 * (i + 1)].reshape(HL, 64).T, np.float32
        )
        bk = wk_b[64 * i:64 * (i + 1)].reshape(64, 1).astype(np.float32)
        bv = wv_b[64 * i:64 * (i + 1)].reshape(64, 1).astype(np.float32)
        woT = np.ascontiguousarray(wo_w[:, 512 * i:512 * (i + 1)].T).astype(BF)
        wob8 = np.zeros((128, MBN), np.float32)
        for mb in range(MBN):
            piece = wo_b[128 * mb:128 * (mb + 1)] / 8.0
            wob8[: len(piece), mb] = piece
        esinkS = np.exp(sinks[HL * i:HL * (i + 1)]).reshape(1, HL).astype(np.float32)
        in_maps.append(
            {
                "xT": xT,
                "wT": wT,
                "bq": bq,
                "bk": bk,
                "bv": bv,
                "cosb": cosb,
                "sinbs": sinbs,
                "woT": woT,
                "wob8": np.ascontiguousarray(wob8),
                "esinkS": esinkS,
            }
        )
    return in_maps


_CACHE = {}


def kernel(**inputs):
    _install_hooks()
    from concourse import bass_utils

    trace = bool(int(os.environ.get("BASS_KERNEL_TRACE", "0")))
    if "nc" not in _CACHE:
        _CACHE["nc"] = build_graph()
    nc = _CACHE["nc"]

    in_maps = make_in_maps(**{k: np.asarray(v) for k, v in inputs.items()})
    res = bass_utils.run_bass_kernel_spmd(
        nc, in_maps, core_ids=list(range(CORES)), trace=trace
    )
    _EXEC_TIME_NS[0] = res.exec_time_ns

    y = np.empty((S, HID), np.float32)
    for i in range(CORES):
        for qc in range(2):
            o = np.asarray(res.results[i][f"o{qc}"], dtype=np.float32)
            y[qc * QC:(qc + 1) * QC, 360 * i:360 * (i + 1)] = o.T
        o2a = np.asarray(res.results[i]["o2a"], dtype=np.float32)
        o2b = np.asarray(res.results[i]["o2b"], dtype=np.float32)
        y[2 * QC:3 * QC, 176 * i:176 * (i + 1)] = o2a.T
        y[2 * QC:3 * QC, 1408 + 184 * i:1408 + 184 * (i + 1)] = o2b.T
    return y.reshape(1, S, HID)


def last_exec_time_ns():
    return _EXEC_TIME_NS[0]
